# revision 46
# baseline (speedup 1.0000x reference)
"""Trainium2 Bass kernel for nn_Balancer (weighted box-mask loss reduction).

reference semantics:
    fg_mask(b,h,w) = union over 32 boxes of [floor(y1)<=h<ceil(y2)] & [floor(x1)<=w<ceil(x2)]
    out = sum(loss * where(fg_mask, 13, 1)) / (B*H*W)

Strategy (data-parallel over batch, 8 cores, 2 images/core):
  - separable box membership: row_in (boxes x 768) and col_in (boxes x 2048)
    computed on-chip from raw f32 coords (integer-grid compares need no
    floor/ceil: h >= floor(y1) <=> h > y1-1, and h < ceil(y2) <=> h < y2).
  - per 128x1024 tile, overlap counts via bf16 matmuls (K = 32 boxes + 2
    delta rows whose products sum to 1/13), so q = count + 1/13 and the
    per-pixel weight is min(q, 1) in {1/13, 1} -- x13 on the host gives
    {1, 13}. The entire weighted sum then needs exactly ONE fused DVE op
    per chunk: scalar_tensor_tensor junk=(q min 1)*loss, accum=row sums.
  - per-core partial sum returned as (1,1); host combines in f64.

Engine budget per core (measured): DMA 12.6MB ~36-39us (roofline),
DVE ~30us, PE ~15us, ACT idle -> DMA-bound.
"""
import numpy as np
from contextlib import ExitStack

import concourse.bass as bass
import concourse.mybir as mybir
import concourse.tile as tile
import concourse.bacc as bacc
from concourse.bass_utils import run_bass_kernel_spmd

B, H, W = 16, 768, 2048
N_CORES = 8
IMGS = B // N_CORES          # images per core = 2
N_PER_IMG = 32
NB = IMGS * N_PER_IMG        # boxes per core = 64
FG_WEIGHT, BG_WEIGHT = 13.0, 1.0
P = 128                      # partitions per row tile
TILES_PER_IMG = H // P       # 6
ROW_TILES = IMGS * TILES_PER_IMG  # 12
MM_N = 512                   # matmul free-dim limit (one PSUM bank, f32)
CHUNK = 2048                 # DVE chunk (4 PSUM banks)
TAIL_CHUNK = 512             # finer chunks on the last row tile -> short tail
# chunk widths per row tile (last tile tapers so the final DVE op after the
# last DMA bytes is only 512 wide, with minimal extra op/drain count)
_CHUNKS = [[CHUNK] * (W // CHUNK)] * (ROW_TILES - 1) + \
          [[1024, 512, 512]]
TOTAL_CHUNKS = sum(len(c) for c in _CHUNKS)  # 11 + 3 = 14
K_MM = N_PER_IMG + 1         # 32 boxes + 1 delta row
IMG_BASE = (0, 64)           # partition base per image (matmul quadrant rule)

f32 = mybir.dt.float32
bf16 = mybir.dt.bfloat16

# Exact-weight trick: delta row adds DELTA=2^-4 to every overlap count, so
# q in {1/16} U [1+1/16, inf). min(q, CAP=13/16) gives {1/16, 13/16}; the
# host multiplies by SCALE=16 -> weights {1, 13} with NO rounding error
# (all three constants are exact in bf16/f32).
DELTA = 0.0625
CAP = 0.8125
SCALE = 16.0

_compiled = {}


def _build(n_reps=1, mode="full", body_reps=1):
    """Build+compile the per-core program. n_reps>1 repeats the pass in a
    For_i loop (timing only). mode: "full" | "dma" | "nostt" | "nomm" |
    "nopsum" -- ablations for bottleneck hunting (results invalid except
    full*). A "2" suffix (full2/dma2) alternates loss DMAs between the
    sync and scalar HWDGE rings."""
    key = (n_reps, mode, body_reps)
    if key in _compiled:
        return _compiled[key]
    dual_ring = mode in ("full2", "dma2")
    if dual_ring:
        mode = mode[:-1]
    # "splitN": N of the 11 full row tiles take the gpsimd weighting path
    # (ACT copies the overlap PSUM->SBUF bf16, gpsimd does min*loss+accum),
    # shrinking the DVE span (each DVE op pays a pipeline DRAIN on top of
    # its stream time, so 16 ops/pass sit right at the DMA span).
    gp_tiles = ()
    if mode.startswith("split"):
        n_gp = int(mode[5:])
        gp_tiles = tuple(range(1, ROW_TILES - 1, max(1, (ROW_TILES - 1) // n_gp)))[:n_gp]
        mode = "full"
    # "wmask": rebuild the box masks inside every rep -- slope(wmask) -
    # slope(full) measures the non-hidden serial cost of mask building,
    # i.e. what a single graded pass pays in startup.
    in_loop_masks = mode == "wmask"
    if in_loop_masks:
        mode = "full"

    nc = bacc.Bacc("TRN2", target_bir_lowering=False, debug=False,
                   num_devices=N_CORES)

    loss_d = nc.dram_tensor("loss", [IMGS * H, W], f32, kind="ExternalInput").ap()
    boxes_d = nc.dram_tensor("boxes", [NB, 4], f32, kind="ExternalInput").ap()
    # raw per-(partition, chunk) accumulator columns; host does the final
    # f64 reduction (removes 3 serial tail ops + a PSUM dependency)
    out_d = nc.dram_tensor("out", [P, TOTAL_CHUNKS], f32, kind="ExternalOutput").ap()

    with tile.TileContext(nc) as tc, ExitStack() as ctx:
        const = ctx.enter_context(tc.tile_pool(name="const", bufs=1))
        # all 12 row tiles resident -> every loss DMA can issue immediately
        lpool = ctx.enter_context(tc.tile_pool(name="loss", bufs=ROW_TILES))
        jpool = ctx.enter_context(tc.tile_pool(name="junk", bufs=2))
        ppool = ctx.enter_context(tc.tile_pool(name="psum", bufs=2, space="PSUM"))
        if gp_tiles:
            spool = ctx.enter_context(tc.tile_pool(name="ovs", bufs=2))
            gpool = ctx.enter_context(tc.tile_pool(name="junkg", bufs=2))

        # --- box membership tensors ---
        # partition layout: img0 boxes at 0..31 (+delta row 32),
        #                   img1 boxes at 64..95 (+delta row 96)
        bx = const.tile([P, 4], f32)
        u1m = const.tile([P, 1], f32)   # x1 - 1
        v1m = const.tile([P, 1], f32)   # y1 - 1
        idx = const.tile([P, W], f32)   # 0..W-1 ramp on every partition
        tmp_r = const.tile([P, H], f32)
        row_in = const.tile([P, H], bf16)
        tmp_c = const.tile([P, W], f32)
        col_in = const.tile([P, W], bf16)

        def build_masks():
            for i in range(IMGS):
                nc.sync.dma_start(bx[IMG_BASE[i]:IMG_BASE[i] + N_PER_IMG, :],
                                  boxes_d[i * N_PER_IMG:(i + 1) * N_PER_IMG, :])
            nc.vector.tensor_scalar(u1m[:], bx[:, 0:1], 1.0, None,
                                    mybir.AluOpType.subtract)
            nc.vector.tensor_scalar(v1m[:], bx[:, 1:2], 1.0, None,
                                    mybir.AluOpType.subtract)
            nc.gpsimd.iota(idx[:], pattern=[[1, W]], base=0, channel_multiplier=0,
                           allow_small_or_imprecise_dtypes=True)
            # first compare on gpsimd (otherwise idle), finisher on DVE;
            # garbage in unused partitions is never read by the matmuls.
            nc.gpsimd.tensor_scalar(tmp_r[:], idx[:, :H], v1m[:], None,
                                    mybir.AluOpType.is_gt)
            nc.vector.scalar_tensor_tensor(row_in[:], idx[:, :H], bx[:, 3:4], tmp_r[:],
                                           mybir.AluOpType.is_lt, mybir.AluOpType.mult)
            # col membership in halves: the first 1024 columns become ready
            # earlier, unblocking tile 0's first matmuls sooner
            for h0 in range(0, W, W // 2):
                hs = slice(h0, h0 + W // 2)
                nc.gpsimd.tensor_scalar(tmp_c[:, hs], idx[:, hs], u1m[:], None,
                                        mybir.AluOpType.is_gt)
                nc.vector.scalar_tensor_tensor(col_in[:, hs], idx[:, hs],
                                               bx[:, 2:3], tmp_c[:, hs],
                                               mybir.AluOpType.is_lt,
                                               mybir.AluOpType.mult)
                for base in IMG_BASE:
                    d0 = base + N_PER_IMG
                    nc.vector.tensor_scalar(col_in[d0:d0 + 1, hs],
                                            idx[d0:d0 + 1, hs], 0.0, 1.0,
                                            mybir.AluOpType.mult,
                                            mybir.AluOpType.add)
            # row delta row per image (col deltas are written per half
            # above). bf16 memset is rejected by the BIR verifier ->
            # (in*0)+c. Partition starts must be quadrant-aligned: 32/96 ok.
            for base in IMG_BASE:
                d0 = base + N_PER_IMG
                nc.vector.tensor_scalar(row_in[d0:d0 + 1, :],
                                        idx[d0:d0 + 1, :H], 0.0, DELTA,
                                        mybir.AluOpType.mult, mybir.AluOpType.add)

        if not in_loop_masks:
            build_masks()

        # --- accumulator (one column per chunk) ---
        macc = const.tile([P, TOTAL_CHUNKS], f32)  # weighted-loss row sums

        # --- main streaming loop ---
        # n_reps>1 (timing only): dynamic loop repeating the identical pass,
        # so device time per dispatch swamps the ~ms axon dispatch cost.
        import contextlib
        rep_cm = (tc.For_i(0, n_reps, 1, staggered_reset=True)
                  if n_reps > 1 else contextlib.nullcontext())
        with rep_cm:
          for rt in range(ROW_TILES * body_reps):
            if in_loop_masks and rt % ROW_TILES == 0:
                build_masks()
            rt = rt % ROW_TILES
            img = rt // TILES_PER_IMG
            tir = rt % TILES_PER_IMG
            bsel = slice(IMG_BASE[img], IMG_BASE[img] + K_MM)

            chunks = _CHUNKS[rt]
            lt = lpool.tile([P, W], f32)
            c0 = 0
            for ci, ch in enumerate(chunks):
                # per-chunk DMA: one 2048-wide contiguous 1MB transfer for
                # full tiles; 4 strided 512-wide ones for the tail tile
                deng = (nc.scalar if dual_ring and (rt + ci) % 2 else nc.sync)
                deng.dma_start(lt[:, c0:c0 + ch],
                               loss_d[rt * P:(rt + 1) * P, c0:c0 + ch])
                c0 += ch

            k0 = sum(len(c) for c in _CHUNKS[:rt])
            c0 = 0
            for cc, ch in enumerate(chunks):
                if mode == "dma":
                    c0 += ch
                    continue
                k = k0 + cc
                lchunk = lt[:, c0:c0 + ch]
                do_mm = mode not in ("nomm", "nopsum")
                if do_mm:
                    ov = ppool.tile([P, ch], f32, tag="ov")
                    for mm in range(ch // MM_N):
                        nc.tensor.matmul(ov[:, mm * MM_N:(mm + 1) * MM_N],
                                         row_in[bsel, tir * P:(tir + 1) * P],
                                         col_in[bsel, c0 + mm * MM_N:c0 + (mm + 1) * MM_N],
                                         start=True, stop=True)
                if mode != "nostt":
                    # q = count + 1/16 ; weight = min(q, 13/16) in {1/16, 13/16}
                    if rt in gp_tiles and do_mm:
                        # gpsimd path: ACT stages q in SBUF (bf16 is safe:
                        # delta stays exact, fg values stay > CAP)
                        ovs = spool.tile([P, ch], bf16, tag="ovs")
                        nc.scalar.activation(ovs[:], ov[:],
                                             mybir.ActivationFunctionType.Copy)
                        junkg = gpool.tile([P, ch], f32, tag="junkg")
                        nc.gpsimd.scalar_tensor_tensor(
                            junkg[:], ovs[:], CAP, lchunk,
                            mybir.AluOpType.min, mybir.AluOpType.mult,
                            accum_out=macc[:, k:k + 1])
                    else:
                        junk = jpool.tile([P, ch], f32, tag="junk")
                        stt_in0 = ov[:] if do_mm else lchunk
                        nc.vector.scalar_tensor_tensor(
                            junk[:], stt_in0, CAP, lchunk,
                            mybir.AluOpType.min, mybir.AluOpType.mult,
                            accum_out=macc[:, k:k + 1])
                c0 += ch

        # --- writeback: raw accumulator columns; host reduces in f64 ---
        if mode in ("dma", "nostt"):
            nc.vector.memset(macc[:], 0.0)
        nc.sync.dma_start(out_d[:], macc[:])

    nc.compile()
    _compiled[key] = nc
    return nc


def _run(loss, gt_boxes2d, trace=False, n_reps=1, mode="full", **kw):
    nc = _build(n_reps, mode)
    loss = np.ascontiguousarray(np.asarray(loss, dtype=np.float32))
    boxes = np.ascontiguousarray(np.asarray(gt_boxes2d, dtype=np.float32))
    in_maps = [
        {"loss": loss[c * IMGS:(c + 1) * IMGS].reshape(IMGS * H, W),
         "boxes": boxes[c * NB:(c + 1) * NB]}
        for c in range(N_CORES)
    ]
    return run_bass_kernel_spmd(nc, in_maps, list(range(N_CORES)), trace=trace, **kw)


def kernel(loss, gt_boxes2d, num_gt_per_img=N_PER_IMG):
    r = _run(loss, gt_boxes2d)
    s = 0.0
    for c in range(N_CORES):
        s += float(np.sum(r.results[c]["out"], dtype=np.float64))
    val = SCALE * s / float(B * H * W)
    return np.float32(val)
